# revision 14
# baseline (speedup 1.0000x reference)
"""FFTEmbedding kernel for Trainium2 (8 NeuronCores, SPMD data-parallel over B).

Math: per (b, t): out = rfft(x_pad[b, t:t+W]) projected by weight + bias.
Linear in x, so it collapses to a causal conv with M2[w, e] (256, 512):
    out[b, t, e] = sum_w x_pad[b, t+w] * M2[w, e] + bias[e]

v2 design (per core: 2 batch rows, weights replicated):
  * WEIGHT-STATIONARY orientation: out tile = [e_blk 128, t 512] in PSUM.
    lhsT = M2 block [w 128, e 128] (8 distinct tiles), rhs = Hankel slice
    [w 128, t 512].  Hank[p, c] = x_pad[b, p + c] (mega-Hankel SBUF image).
  * [e, t] layout enables SINGLE-PASS evacuation with the bias fused as a
    per-partition vector: ACT activation(Identity, bias=AP) and DVE
    tensor_scalar(add, AP) both do PSUM->SBUF + bias + fp16 cast in one op.
    Evacuations are paired [128, 1024] (2 banks, segs s/s+1) and split
    between DVE (eb 0,1) and ACT (eb 2,3) - each engine ~35-38us << PE 55us.
  * Loop: row-outer, then 8 seg-pairs of 1024 t, then 4 e-blocks. PSUM =
    4 x [128, 1024] tiles = all 8 banks, recycled per seg-pair.
  * Output DRAM layout is [b, e, t] (host transposes back): per (row, eb)
    the sup tile [128, 8192] fp16 DMAs out in contiguous 2048-col waves
    (4 KB runs/partition vs 1 KB in v1 - much better DMA efficiency).
  * Hankel build: chunk0/1 (t<1536) load direct from HBM (128 shifted
    reads). Chunk2 (t in [1536, 8192)) loads only partitions 0:32 from HBM
    (stage1), then 3 SBUF->SBUF DMA copies replicate to partitions 32:128
    with col shifts (stage2, SWDGE) - cuts the redundant HBM read ~4x.
  * PE warm-up: HAM clock gate needs ~3.4us of sustained PE activity; junk
    matmuls (vs memset tile) start right at user-program start so the real
    MM stream runs at the warm 2.4 GHz rate (~216 ns / N=512 MM).
  * Output stored fp16 ([b, e, t]); host transposes to [b, t, e] and
    upcasts to fp32. Measured end-to-end rel err ~4e-4.
"""

import os
import sys

import numpy as np

_TRN_REPO = "/opt/trn_rl_repo"
if _TRN_REPO not in sys.path:
    sys.path.insert(0, _TRN_REPO)

B, T, W_SIZE, EMB = 16, 8192, 256, 512
N_CORES = 8
B_PER = B // N_CORES          # 2 batch rows per core
PAD = W_SIZE - 1              # 255 leading zeros
XP_LEN = T + PAD + 1          # 8448 (one trailing pad elem)

# t-space chunks of the Hankel image per row; chunk j covers t in
# [OFF[j], OFF[j+1]).  Boundaries must be multiples of 512.
CHUNKS = [1024, 1536, 5632]
OFF = [0, 1024, 2560, 8192]
# stage parameter k per (row, chunk): 128 = direct HBM load of all 128
# partitions; k<128 = load partitions [0:k] from HBM (stage1), then
# (128/k - 1) SBUF->SBUF copies with col shifts (stage2) on HWDGE rings
# (SWDGE/gpsimd delivers its data ~10us late - never use it here).
# row0 c0/c1 gate the matmul stream -> direct; row0 c2 k=64 (1 copy);
# row1 has ~30us slack -> k=32 (3 copies), loads deferred into the loop.
KSTAGE = {0: [128, 128, 64], 1: [32, 32, 32]}

N_SEG = T // 512              # 16 segs of 512 t per row
N_SP = N_SEG // 2             # 8 seg-pairs of 1024 t

TRACE = os.environ.get("KERNEL_TRACE", "0") == "1"
N_WARM = int(os.environ.get("KERNEL_WARM", "8"))
LAST_RESULT = None

_CACHE = {}


def _build_m2(weight: np.ndarray) -> np.ndarray:
    """(EMB, 258) projection -> (W, EMB) causal-conv matrix, in float64."""
    k = np.arange(W_SIZE // 2 + 1, dtype=np.float64)   # 129
    w = np.arange(W_SIZE, dtype=np.float64)            # 256
    ang = 2.0 * np.pi * np.outer(k, w) / W_SIZE        # (129, 256)
    f = np.concatenate([np.cos(ang), -np.sin(ang)], axis=0)  # (258, 256)
    m2 = (weight.astype(np.float64) @ f).T             # (256, EMB)
    return np.ascontiguousarray(m2, dtype=np.float64)


def _build_program():
    from concourse import bacc, mybir, tile
    from concourse.ap import AP

    f32 = mybir.dt.float32
    f16 = mybir.dt.float16
    add = mybir.AluOpType.add
    ident = mybir.ActivationFunctionType.Identity

    nc = bacc.Bacc(target_bir_lowering=False)
    xpad_h = nc.declare_dram_parameter("xpad", [B_PER, XP_LEN], f16, isOutput=False)
    # w2 packed on host: w2[p, eb*256 + h*128 + m] = M2[128h + p, 128eb + m]
    w2_h = nc.declare_dram_parameter("w2", [128, 2 * EMB], f16, isOutput=False)
    # bias4[p, eb] = bias[128eb + p]
    bias4_h = nc.declare_dram_parameter("bias4", [128, 4], f32, isOutput=False)
    out_h = nc.declare_dram_parameter("out", [B_PER, EMB, T], f16, isOutput=True)

    with tile.TileContext(nc) as tc:
        with (
            tc.tile_pool(name="hank", bufs=1) as hank_pool,
            tc.tile_pool(name="wpool", bufs=1) as w_pool,
            tc.tile_pool(name="cpool", bufs=1) as c_pool,
            tc.tile_pool(name="sup", bufs=1) as sup_pool,
            tc.tile_pool(name="psum", bufs=4, space="PSUM") as psum_pool,
        ):
            # ---- PE warm-up: junk matmuls with no input dependency ----
            # memset on gpsimd (otherwise idle); DVE stays clear for evacs
            junk = c_pool.tile([128, 512], f16, tag="junk")
            nc.gpsimd.memset(junk[:, :], 0.0)
            ps_warm = psum_pool.tile([128, 2 * EMB], f32, name="ps_warm", tag="ps")
            for _ in range(N_WARM):
                nc.tensor.matmul(
                    ps_warm[:, 0:EMB], junk[:, 0:128], junk[:, :],
                    start=True, stop=True,
                )

            # ---- constants / weights ----
            w01 = w_pool.tile([128, 2 * EMB], f16, tag="w01")
            nc.scalar.dma_start(w01[:, :], w2_h[:, :])
            bias4 = c_pool.tile([128, 4], f32, tag="bias4")

            def wslice(h, eb):
                lo = eb * 256 + h * 128
                return w01[:, lo : lo + 128]

            # ---- Hankel images (one per batch row) ----
            # tile width: len + 128 (h=1 reach); staged chunks +96 more so
            # stage2 copies read within the tile.
            hank = [[None] * len(CHUNKS) for _ in range(B_PER)]

            def make_chunk_tiles(b):
                for j, ln in enumerate(CHUNKS):
                    k = KSTAGE[b][j]
                    w = ln + 128 + (128 - k)
                    hank[b][j] = hank_pool.tile(
                        [128, w], f16, tag=f"hk{j}_{b}", name=f"hk{j}_{b}"
                    )

            def stage1(b, j, eng):
                t = hank[b][j]
                k = KSTAGE[b][j]
                base = b * XP_LEN + OFF[j]
                cols1 = CHUNKS[j] + 128 + (128 - k)
                eng.dma_start(
                    t[0:k, :cols1], AP(xpad_h, base, [[1, k], [1, cols1]])
                )

            def stage2(b, j, eng):
                t = hank[b][j]
                k = KSTAGE[b][j]
                cols2 = CHUNKS[j] + 128
                for m in range(1, 128 // k):
                    eng.dma_start(
                        t[k * m : k * (m + 1), 0:cols2],
                        t[0:k, k * m : k * m + cols2],
                    )

            make_chunk_tiles(0)
            make_chunk_tiles(1)
            # LEAN critical path: only what gates the first matmuls moves
            # early (w2 on scalar; row0 c0/c1 on sync).  Everything else is
            # deferred so it doesn't steal HBM bandwidth from the stream
            # start (completion sems scale with total bytes in flight).
            stage1(0, 0, nc.sync)      # direct [128, 1152]
            stage1(0, 1, nc.sync)      # direct [128, 1664]
            stage1(0, 2, nc.scalar)    # k=64 stage1, after w2
            nc.scalar.dma_start(bias4[:, :], bias4_h[:, :])
            stage2(0, 2, nc.sync)      # 1 copy, waits stage1(0,2)

            def rhs(b, t0, h):
                """Hankel slice [w 128, t 512] for seg at t0, K-half h."""
                for j in range(len(CHUNKS)):
                    if t0 < OFF[j + 1]:
                        c0 = t0 - OFF[j] + 128 * h
                        return hank[b][j][:, c0 : c0 + 512]
                raise AssertionError(t0)

            # ---- sup (output staging) tiles ----
            sup = [
                [
                    sup_pool.tile([128, T], f16, tag=f"sup{b}_{eb}", name=f"sup{b}_{eb}")
                    for eb in range(4)
                ]
                for b in range(B_PER)
            ]

            # ---- main loop ----
            def out_dma(eng, b, eb, lo, hi):
                eng.dma_start(
                    out_h[b, eb * 128 : (eb + 1) * 128, lo:hi],
                    sup[b][eb][:, lo:hi],
                )

            for b in range(B_PER):
                for sp in range(N_SP):
                    t0 = 1024 * sp
                    # the kernel's final seg-pair gets fine-grained (per-seg)
                    # evacuation + immediate per-eb out-DMA to shorten the tail
                    last_sp = b == B_PER - 1 and sp == N_SP - 1
                    for eb in range(4):
                        ps = psum_pool.tile(
                            [128, 2 * EMB], f32, name=f"ps_{b}_{sp}_{eb}", tag="ps"
                        )
                        bvec = bias4[:, eb : eb + 1]
                        for s in range(2):
                            pslice = ps[:, s * 512 : (s + 1) * 512]
                            nc.tensor.matmul(
                                pslice, wslice(0, eb), rhs(b, t0 + 512 * s, 0),
                                start=True, stop=False,
                            )
                            nc.tensor.matmul(
                                pslice, wslice(1, eb), rhs(b, t0 + 512 * s, 1),
                                start=False, stop=True,
                            )
                            if last_sp:
                                dst = sup[b][eb][:, t0 + 512 * s : t0 + 512 * (s + 1)]
                                if eb < 2:
                                    nc.vector.tensor_scalar(dst, pslice, bvec, None, add)
                                else:
                                    nc.scalar.activation(dst, pslice, ident, bias=bvec)
                                out_dma(
                                    nc.scalar if eb < 2 else nc.sync,
                                    b, eb, t0 + 512 * s, t0 + 512 * (s + 1),
                                )
                        if not last_sp:
                            dst = sup[b][eb][:, t0 : t0 + 1024]
                            if eb < 2:
                                nc.vector.tensor_scalar(dst, ps[:, :], bvec, None, add)
                            else:
                                nc.scalar.activation(dst, ps[:, :], ident, bias=bvec)
                    # out-DMA waves: big 2048-col waves early, per-seg-pair
                    # 1024-col waves from sp4 on (keeps the drain smooth);
                    # late row1 waves put eb0 on the scalar ring to cut the
                    # single-queue issue serialization at the tail.
                    if not last_sp:
                        if sp in (1, 3):
                            for eb in range(4):
                                out_dma(nc.sync, b, eb, t0 - 1024, t0 + 1024)
                        elif sp >= 4:
                            for eb in range(4):
                                eng = (
                                    nc.scalar
                                    if (b == B_PER - 1 and eb == 0)
                                    else nc.sync
                                )
                                out_dma(eng, b, eb, t0, t0 + 1024)
                    # deferred input loads for row1, placed where they can't
                    # steal bandwidth from the stream start
                    if b == 0 and sp == 1:
                        for j in range(3):
                            stage1(1, j, nc.scalar)
                    if b == 0 and sp == 2:
                        for j in range(3):
                            stage2(1, j, nc.sync)

    nc.finalize()
    return nc


def _get_program():
    if "prog" not in _CACHE:
        _CACHE["prog"] = _build_program()
    return _CACHE["prog"]


def kernel(x: np.ndarray, weight: np.ndarray, bias: np.ndarray) -> np.ndarray:
    global LAST_RESULT
    from concourse.bass_utils import run_bass_kernel_spmd

    x = np.asarray(x, dtype=np.float32)
    weight = np.asarray(weight, dtype=np.float32)
    bias = np.asarray(bias, dtype=np.float32)

    m2 = _build_m2(weight)
    xpad = np.zeros((B, XP_LEN), dtype=np.float32)
    xpad[:, PAD : PAD + T] = x
    # w2[p, eb*256 + h*128 + m] = M2[128h + p, 128eb + m]
    w2_in = np.ascontiguousarray(
        m2.reshape(2, 128, 4, 128).transpose(1, 2, 0, 3).reshape(128, 2 * EMB)
    ).astype(np.float16)
    bias4 = np.ascontiguousarray(bias.reshape(4, 128).T).astype(np.float32)
    xpad16 = xpad.astype(np.float16)

    nc = _get_program()
    in_maps = [
        {
            "xpad": np.ascontiguousarray(xpad16[c * B_PER : (c + 1) * B_PER]),
            "w2": w2_in,
            "bias4": bias4,
        }
        for c in range(N_CORES)
    ]
    res = run_bass_kernel_spmd(nc, in_maps, list(range(N_CORES)), trace=TRACE)
    LAST_RESULT = res
    out_bet = np.concatenate(
        [res.results[c]["out"] for c in range(N_CORES)], axis=0
    )  # (B, EMB, T) fp16
    out = out_bet.transpose(0, 2, 1).astype(np.float32)
    return np.ascontiguousarray(out)


# revision 18
# speedup vs baseline: 1.2143x; 1.2143x over previous
"""FFTEmbedding kernel for Trainium2 (8 NeuronCores, SPMD data-parallel over B).

Math: per (b, t): out = rfft(x_pad[b, t:t+W]) projected by weight + bias.
Linear in x, so it collapses to a causal conv with M2[w, e] (256, 512):
    out[b, t, e] = sum_w x_pad[b, t+w] * M2[w, e] + bias[e]

v2 design (per core: 2 batch rows, weights replicated):
  * WEIGHT-STATIONARY orientation: out tile = [e_blk 128, t 512] in PSUM.
    lhsT = M2 block [w 128, e 128] (8 distinct tiles), rhs = Hankel slice
    [w 128, t 512].  Hank[p, c] = x_pad[b, p + c] (mega-Hankel SBUF image).
  * [e, t] layout enables SINGLE-PASS evacuation with the bias fused as a
    per-partition vector: ACT activation(Identity, bias=AP) and DVE
    tensor_scalar(add, AP) both do PSUM->SBUF + bias + fp16 cast in one op.
    Evacuations are paired [128, 1024] (2 banks, segs s/s+1) and split
    between DVE (eb 0,1) and ACT (eb 2,3) - each engine ~35-38us << PE 55us.
  * Loop: row-outer, then 8 seg-pairs of 1024 t, then 4 e-blocks. PSUM =
    4 x [128, 1024] tiles = all 8 banks, recycled per seg-pair.
  * Output DRAM layout is [b, e, t] (host transposes back): per (row, eb)
    the sup tile [128, 8192] fp16 DMAs out in contiguous 2048-col waves
    (4 KB runs/partition vs 1 KB in v1 - much better DMA efficiency).
  * Hankel build: chunk0/1 (t<1536) load direct from HBM (128 shifted
    reads). Chunk2 (t in [1536, 8192)) loads only partitions 0:32 from HBM
    (stage1), then 3 SBUF->SBUF DMA copies replicate to partitions 32:128
    with col shifts (stage2, SWDGE) - cuts the redundant HBM read ~4x.
  * PE warm-up: HAM clock gate needs ~3.4us of sustained PE activity; junk
    matmuls (vs memset tile) start right at user-program start so the real
    MM stream runs at the warm 2.4 GHz rate (~216 ns / N=512 MM).
  * Output stored fp16 ([b, e, t]); host transposes to [b, t, e] and
    upcasts to fp32. Measured end-to-end rel err ~4e-4.
"""

import os
import sys

import numpy as np

_TRN_REPO = "/opt/trn_rl_repo"
if _TRN_REPO not in sys.path:
    sys.path.insert(0, _TRN_REPO)

B, T, W_SIZE, EMB = 16, 8192, 256, 512
N_CORES = 8
B_PER = B // N_CORES          # 2 batch rows per core
PAD = W_SIZE - 1              # 255 leading zeros
XP_LEN = T + PAD + 1          # 8448 (one trailing pad elem)

# t-space chunks of the Hankel image per row; chunk j covers t in
# [OFF[j], OFF[j+1]).  Boundaries must be multiples of 512.
CHUNKS = [1024, 2560, 4608]
OFF = [0, 1024, 3584, 8192]
# stage parameter k per (row, chunk): 128 = direct HBM load of all 128
# partitions; k<128 = load partitions [0:k] from HBM (stage1), then
# (128/k - 1) SBUF->SBUF copies with col shifts (stage2) on HWDGE rings
# (SWDGE/gpsimd delivers its data ~10us late - only OK for non-critical).
# row0 gates the matmul stream -> all direct (c2 deferred into the loop);
# row1 has ~30us slack -> k=32 (3 copies), loads deferred into the loop.
KSTAGE = {0: [128, 128, 128], 1: [32, 32, 32]}

N_SEG = T // 512              # 16 segs of 512 t per row
N_SP = N_SEG // 2             # 8 seg-pairs of 1024 t

TRACE = os.environ.get("KERNEL_TRACE", "0") == "1"
N_WARM = int(os.environ.get("KERNEL_WARM", "8"))
LAST_RESULT = None

_CACHE = {}


def _build_m2(weight: np.ndarray) -> np.ndarray:
    """(EMB, 258) projection -> (W, EMB) causal-conv matrix, in float64."""
    k = np.arange(W_SIZE // 2 + 1, dtype=np.float64)   # 129
    w = np.arange(W_SIZE, dtype=np.float64)            # 256
    ang = 2.0 * np.pi * np.outer(k, w) / W_SIZE        # (129, 256)
    f = np.concatenate([np.cos(ang), -np.sin(ang)], axis=0)  # (258, 256)
    m2 = (weight.astype(np.float64) @ f).T             # (256, EMB)
    return np.ascontiguousarray(m2, dtype=np.float64)


def _build_program():
    from concourse import bacc, mybir, tile
    from concourse.ap import AP

    f32 = mybir.dt.float32
    f16 = mybir.dt.float16
    add = mybir.AluOpType.add
    ident = mybir.ActivationFunctionType.Identity

    nc = bacc.Bacc(target_bir_lowering=False)
    xpad_h = nc.declare_dram_parameter("xpad", [B_PER, XP_LEN], f16, isOutput=False)
    # w2 packed on host: w2[p, eb*256 + h*128 + m] = M2[128h + p, 128eb + m]
    w2_h = nc.declare_dram_parameter("w2", [128, 2 * EMB], f16, isOutput=False)
    # bias4[p, eb] = bias[128eb + p]
    bias4_h = nc.declare_dram_parameter("bias4", [128, 4], f32, isOutput=False)
    out_h = nc.declare_dram_parameter("out", [B_PER, EMB, T], f16, isOutput=True)

    with tile.TileContext(nc) as tc:
        with (
            tc.tile_pool(name="hank", bufs=1) as hank_pool,
            tc.tile_pool(name="wpool", bufs=1) as w_pool,
            tc.tile_pool(name="cpool", bufs=1) as c_pool,
            tc.tile_pool(name="sup", bufs=1) as sup_pool,
            tc.tile_pool(name="psum", bufs=4, space="PSUM") as psum_pool,
        ):
            # ---- PE warm-up: junk matmuls with no input dependency ----
            # memset on gpsimd (otherwise idle); DVE stays clear for evacs
            junk = c_pool.tile([128, 512], f16, tag="junk")
            nc.gpsimd.memset(junk[:, :], 0.0)
            ps_warm = psum_pool.tile([128, 2 * EMB], f32, name="ps_warm", tag="ps")
            for _ in range(N_WARM):
                nc.tensor.matmul(
                    ps_warm[:, 0:EMB], junk[:, 0:128], junk[:, :],
                    start=True, stop=True,
                )

            # ---- constants / weights ----
            w01 = w_pool.tile([128, 2 * EMB], f16, tag="w01")
            nc.scalar.dma_start(w01[:, :], w2_h[:, :])
            bias4 = c_pool.tile([128, 4], f32, tag="bias4")

            def wslice(h, eb):
                lo = eb * 256 + h * 128
                return w01[:, lo : lo + 128]

            # ---- Hankel images (one per batch row) ----
            # tile width: len + 128 (h=1 reach); staged chunks +96 more so
            # stage2 copies read within the tile.
            hank = [[None] * len(CHUNKS) for _ in range(B_PER)]

            def make_chunk_tiles(b):
                for j, ln in enumerate(CHUNKS):
                    k = KSTAGE[b][j]
                    w = ln + 128 + (128 - k)
                    hank[b][j] = hank_pool.tile(
                        [128, w], f16, tag=f"hk{j}_{b}", name=f"hk{j}_{b}"
                    )

            def stage1(b, j, eng):
                t = hank[b][j]
                k = KSTAGE[b][j]
                base = b * XP_LEN + OFF[j]
                cols1 = CHUNKS[j] + 128 + (128 - k)
                eng.dma_start(
                    t[0:k, :cols1], AP(xpad_h, base, [[1, k], [1, cols1]])
                )

            def stage2(b, j, eng):
                t = hank[b][j]
                k = KSTAGE[b][j]
                cols2 = CHUNKS[j] + 128
                for m in range(1, 128 // k):
                    eng.dma_start(
                        t[k * m : k * (m + 1), 0:cols2],
                        t[0:k, k * m : k * m + cols2],
                    )

            make_chunk_tiles(0)
            make_chunk_tiles(1)
            # LEAN critical path: only what gates the first matmuls moves
            # early (w2 on scalar; row0 c0/c1 on sync).  Everything else is
            # deferred so it doesn't steal HBM bandwidth from the stream
            # start (completion sems scale with total bytes in flight).
            stage1(0, 0, nc.sync)      # direct [128, 1152]
            stage1(0, 1, nc.sync)      # direct [128, 2688]
            nc.scalar.dma_start(bias4[:, :], bias4_h[:, :])

            def rhs(b, t0, h):
                """Hankel slice [w 128, t 512] for seg at t0, K-half h."""
                for j in range(len(CHUNKS)):
                    if t0 < OFF[j + 1]:
                        c0 = t0 - OFF[j] + 128 * h
                        return hank[b][j][:, c0 : c0 + 512]
                raise AssertionError(t0)

            # ---- sup (output staging) tiles ----
            sup = [
                [
                    sup_pool.tile([128, T], f16, tag=f"sup{b}_{eb}", name=f"sup{b}_{eb}")
                    for eb in range(4)
                ]
                for b in range(B_PER)
            ]

            # ---- main loop ----
            def out_dma(eng, b, eb, lo, hi):
                eng.dma_start(
                    out_h[b, eb * 128 : (eb + 1) * 128, lo:hi],
                    sup[b][eb][:, lo:hi],
                )

            for b in range(B_PER):
                for sp in range(N_SP):
                    t0 = 1024 * sp
                    # the kernel's final seg-pair gets fine-grained (per-seg)
                    # evacuation + immediate per-eb out-DMA to shorten the tail
                    last_sp = b == B_PER - 1 and sp == N_SP - 1
                    for eb in range(4):
                        ps = psum_pool.tile(
                            [128, 2 * EMB], f32, name=f"ps_{b}_{sp}_{eb}", tag="ps"
                        )
                        bvec = bias4[:, eb : eb + 1]
                        for s in range(2):
                            pslice = ps[:, s * 512 : (s + 1) * 512]
                            nc.tensor.matmul(
                                pslice, wslice(0, eb), rhs(b, t0 + 512 * s, 0),
                                start=True, stop=False,
                            )
                            nc.tensor.matmul(
                                pslice, wslice(1, eb), rhs(b, t0 + 512 * s, 1),
                                start=False, stop=True,
                            )
                            if last_sp:
                                dst = sup[b][eb][:, t0 + 512 * s : t0 + 512 * (s + 1)]
                                if eb < 2:
                                    nc.vector.tensor_scalar(dst, pslice, bvec, None, add)
                                else:
                                    nc.scalar.activation(dst, pslice, ident, bias=bvec)
                                out_dma(
                                    nc.scalar if eb == 0 else nc.sync,
                                    b, eb, t0 + 512 * s, t0 + 512 * (s + 1),
                                )
                        if not last_sp:
                            dst = sup[b][eb][:, t0 : t0 + 1024]
                            if eb < 2:
                                nc.vector.tensor_scalar(dst, ps[:, :], bvec, None, add)
                            else:
                                nc.scalar.activation(dst, ps[:, :], ident, bias=bvec)
                    # out-DMA waves: big 2048-col waves early, per-seg-pair
                    # 1024-col waves from sp4 on (keeps the drain smooth).
                    # Non-tail waves ride the otherwise-idle SWDGE (gpsimd)
                    # ring - its multi-us latency is harmless mid-stream and
                    # it keeps the HWDGE rings free of issue backlog.  Only
                    # the kernel's last few waves (latency-critical for the
                    # finish barrier) use the HWDGE rings, split across both.
                    tail_wave = b == B_PER - 1 and sp >= 5
                    if not last_sp:
                        if sp in (1, 3):
                            for eb in range(4):
                                out_dma(nc.gpsimd, b, eb, t0 - 1024, t0 + 1024)
                        elif sp >= 4:
                            for eb in range(4):
                                if tail_wave:
                                    eng = nc.scalar if eb == 0 else nc.sync
                                else:
                                    eng = nc.gpsimd
                                out_dma(eng, b, eb, t0, t0 + 1024)
                    # deferred input loads, placed where they can't steal
                    # HBM bandwidth from the stream start
                    if b == 0 and sp == 0:
                        stage1(0, 2, nc.scalar)   # direct [128, 4736]
                    if b == 0 and sp == 1:
                        for j in range(3):
                            stage1(1, j, nc.scalar)
                    if b == 0 and sp == 2:
                        for j in range(3):
                            stage2(1, j, nc.sync)

    nc.finalize()
    return nc


def _get_program():
    if "prog" not in _CACHE:
        _CACHE["prog"] = _build_program()
    return _CACHE["prog"]


def kernel(x: np.ndarray, weight: np.ndarray, bias: np.ndarray) -> np.ndarray:
    global LAST_RESULT
    from concourse.bass_utils import run_bass_kernel_spmd

    x = np.asarray(x, dtype=np.float32)
    weight = np.asarray(weight, dtype=np.float32)
    bias = np.asarray(bias, dtype=np.float32)

    m2 = _build_m2(weight)
    xpad = np.zeros((B, XP_LEN), dtype=np.float32)
    xpad[:, PAD : PAD + T] = x
    # w2[p, eb*256 + h*128 + m] = M2[128h + p, 128eb + m]
    w2_in = np.ascontiguousarray(
        m2.reshape(2, 128, 4, 128).transpose(1, 2, 0, 3).reshape(128, 2 * EMB)
    ).astype(np.float16)
    bias4 = np.ascontiguousarray(bias.reshape(4, 128).T).astype(np.float32)
    xpad16 = xpad.astype(np.float16)

    nc = _get_program()
    in_maps = [
        {
            "xpad": np.ascontiguousarray(xpad16[c * B_PER : (c + 1) * B_PER]),
            "w2": w2_in,
            "bias4": bias4,
        }
        for c in range(N_CORES)
    ]
    res = run_bass_kernel_spmd(nc, in_maps, list(range(N_CORES)), trace=TRACE)
    LAST_RESULT = res
    out_bet = np.concatenate(
        [res.results[c]["out"] for c in range(N_CORES)], axis=0
    )  # (B, EMB, T) fp16
    out = out_bet.transpose(0, 2, 1).astype(np.float32)
    return np.ascontiguousarray(out)
